# revision 23
# baseline (speedup 1.0000x reference)
"""Trainium2 Bass kernel for DGL HyperGCNII conv (hypergraph message passing).

Computation (reference):
    Xe = segment_sum(X[g1_src], g1_dst, E) * degE          # nodes -> hyperedges
    Xv = segment_sum(Xe[g2_src], g2_dst, N) * degV         # hyperedges -> nodes
    Xi = (1-a)*Xv + a*X0
    out = (1-b)*Xi + b*(Xi @ W.T)

Strategy (8 NeuronCores, vertex-cut graph parallelism):
- Shard nodes across cores.  Each phase's nnz are globally sorted by
  destination and packed into 128-slot tiles; per-block counts are padded to
  the max across cores so the compiled schedule is core-uniform (SPMD).
- Gathers run as SWDGE dma_gather calls round-robined over the 4 SWDGE
  queues (4 Q7 core-pairs emit descriptors in parallel).  Gather tiles are
  grouped into large rotating ARENAS (4 calls per arena, one per queue) so
  the descriptor rings stay deep; random 256B HBM reads are latency bound,
  so ring depth is what buys aggregate drain throughput.
- Segment-sum via one-hot selection matmuls.  The one-hot S tiles are pure
  index metadata and are precomputed host-side (degE / degV*(1-alpha) folds
  included) and streamed from HBM per block -- building them on DVE/ACT
  on-chip stalls badly on SBUF bank conflicts with the gather drain.
- AllReduce (fp16, 2 chunks, triggered from the Scalar engine so the first
  chunk overlaps the phase-1 gather tail) of Xe partials across 8 cores.
- Phase 2 accumulates transposed (Xv^T), adds a*X0^T, applies
  M = (1-b)I + b*W via a second matmul which also un-transposes, writes out.

All indices / one-hot selection matrices are precomputed host-side as int16 /
f16 metadata (index-only preprocessing); data math happens on device.
"""

import hashlib
import os
import numpy as np
from contextlib import ExitStack
from dataclasses import dataclass

import concourse.bass as bass
import concourse.tile as tile
from concourse import bacc, mybir
from concourse.bass_utils import run_bass_kernel_spmd
from concourse.library_config import mlp

P = 128
F32 = mybir.dt.float32
F16 = mybir.dt.float16
I16 = mybir.dt.int16
NQ = 4   # SWDGE queues (4 Q7 core-pairs)
AT = 48  # tiles per arena (divisible by NQ)
ABUFS = 4


@dataclass(frozen=True)
class Cfg:
    n_nodes: int = 100000
    n_edges: int = 20000
    d: int = 128
    ncores: int = 8
    ar_chunks: int = 1
    wb: int = 8   # blocks per batched DRAM write

    @property
    def nb_v(self):
        return -(-self.n_nodes // (self.ncores * P))

    @property
    def nsh(self):
        return self.nb_v * P

    @property
    def n_pad(self):
        return self.nsh * self.ncores

    @property
    def nb_e(self):
        return -(-self.n_edges // P)

    @property
    def e_pad(self):
        return self.nb_e * P


CFG = Cfg()


def _common_layout(cnts):
    """Uniform (across cores) slot-stream layout from per-block padded counts.

    Returns (off[nblocks+1], T, pairs list of (tile, block), per_block).
    """
    nblocks = len(cnts)
    off = np.zeros(nblocks + 1, np.int64)
    np.cumsum(cnts, out=off[1:])
    S = int(off[-1])
    T = max(1, -(-S // P))
    pairs = []
    per_block = [[] for _ in range(nblocks)]
    for b in range(nblocks):
        if cnts[b] == 0:
            continue
        t0 = int(off[b]) // P
        t1 = int(off[b] + cnts[b] - 1) // P
        for t in range(t0, t1 + 1):
            per_block[b].append(len(pairs))
            pairs.append((t, b))
    return off, T, pairs, per_block


def _fill_core(src, dst_local, colw, slotw, off, T, pairs, nblocks):
    """Place one core's nnz into the common layout.

    colw: per-destination column weights [nblocks*128] (phase 1: degE) or
          None; slotw: per-nnz slot weights (phase 2: degV*(1-a)) or None.
    Returns (idx_slots[T*128] int64, sx [128, npairs*128] f16 one-hot tiles).
    """
    dl = np.asarray(dst_local, np.int64)
    order = np.argsort(dl, kind="stable")
    s = np.asarray(src, np.int64)[order]
    dls = dl[order]
    blk = dls // P
    bc = np.bincount(blk, minlength=nblocks)
    bstart = np.zeros(nblocks + 1, np.int64)
    np.cumsum(bc, out=bstart[1:])
    rank = np.arange(len(dls)) - bstart[blk]
    pos = off[blk] + rank
    BIG = np.int64(1) << 40
    idx_slots = np.zeros(T * P, np.int64)
    dl_full = np.full(T * P, BIG)
    w_full = np.ones(T * P, np.float32)
    idx_slots[pos] = s
    dl_full[pos] = dls
    if slotw is not None:
        w_full[:] = 0.0
        w_full[pos] = np.asarray(slotw, np.float32)[order]
    blk_full = dl_full // P

    npairs = len(pairs)
    tile_of_pair = np.asarray([t for t, _ in pairs], np.int64)
    blk_of_pair = np.asarray([b for _, b in pairs], np.int64)
    sx = np.zeros((npairs, P, P), np.float16)
    slot_mat = dl_full.reshape(T, P)
    blk_mat = blk_full.reshape(T, P)
    w_mat = w_full.reshape(T, P)
    for i in range(npairs):
        t, b = tile_of_pair[i], blk_of_pair[i]
        m = blk_mat[t] == b
        if not m.any():
            continue
        cols = (slot_mat[t][m] - b * P).astype(np.int64)
        vals = w_mat[t][m].astype(np.float32)
        if colw is not None:
            vals = vals * colw[b * P + cols]
        sx[i, np.nonzero(m)[0], cols] = vals.astype(np.float16)
    sx = np.ascontiguousarray(sx.transpose(1, 0, 2).reshape(P, npairs * P))
    return idx_slots, sx


def _pack_idx(idx_slots, T):
    """[T*128] slot ids -> SWDGE 16-wrap [128, T*8] int16."""
    cols = []
    for t in range(T):
        flat = idx_slots[t * P:(t + 1) * P].astype(np.int16)
        wrap = flat.reshape(-1, 16).T          # [16, 8]
        cols.append(np.tile(wrap, (8, 1)))     # [128, 8]
    return np.ascontiguousarray(np.concatenate(cols, axis=1))


def _ar_bounds(cfg):
    NB_E = cfg.nb_e
    if cfg.ar_chunks == 3:
        return [0, round(0.40 * NB_E), round(0.70 * NB_E), NB_E]
    return [round(i * NB_E / cfg.ar_chunks) for i in range(cfg.ar_chunks + 1)]


def _xe_row_of_edge(cfg):
    """Edge id -> row in the chunked [j, b, d] xe layout."""
    bnds = _ar_bounds(cfg)
    e = np.arange(cfg.e_pad, dtype=np.int64)
    b = e // P
    j = e % P
    row = np.zeros(cfg.e_pad, np.int64)
    base = 0
    for c in range(len(bnds) - 1):
        lo, hi = bnds[c], bnds[c + 1]
        w = hi - lo
        m = (b >= lo) & (b < hi)
        row[m] = base + j[m] * w + (b[m] - lo)
        base += P * w
    return row


_PROGRAM_CACHE = {}


def _schedule_hash(sched1, sched2, alpha):
    h = hashlib.sha1()
    for pairs, per_block, T in (sched1, sched2):
        h.update(np.int64(T).tobytes())
        h.update(np.asarray([p for pr in pairs for p in pr], np.int64).tobytes())
        for pb in per_block:
            h.update(np.asarray(pb + [-1], np.int64).tobytes())
    h.update(np.float64(alpha).tobytes())
    return h.hexdigest()


def build_program(sched1, sched2, alpha, cfg=CFG, compile=True):
    key = _schedule_hash(sched1, sched2, alpha)
    if key in _PROGRAM_CACHE:
        return _PROGRAM_CACHE[key]

    D = cfg.d
    NSH, NB_V, NB_E, E_PAD = cfg.nsh, cfg.nb_v, cfg.nb_e, cfg.e_pad
    pairs1, per_block1, T1 = sched1
    pairs2, per_block2, T2 = sched2
    NP1, NP2 = len(pairs1), len(pairs2)
    MAXC1 = max((len(x) for x in per_block1 if x), default=1)
    MAXC2 = max((len(x) for x in per_block2 if x), default=1)
    TPC = AT // NQ

    nc = bacc.Bacc("TRN2", target_bir_lowering=False, debug=False,
                   num_devices=cfg.ncores, num_swdge_queues=NQ)

    xsh = nc.dram_tensor("xsh", [NSH, D], F32, kind="ExternalInput")
    x0t = nc.dram_tensor("x0t", [D, NSH], F32, kind="ExternalInput")
    idx1 = nc.dram_tensor("idx1", [P, T1 * 8], I16, kind="ExternalInput")
    idx2 = nc.dram_tensor("idx2", [P, T2 * 8], I16, kind="ExternalInput")
    s1x = nc.dram_tensor("s1x", [P, NP1 * P], F16, kind="ExternalInput")
    s2x = nc.dram_tensor("s2x", [P, NP2 * P], F16, kind="ExternalInput")
    m_arr = nc.dram_tensor("m_arr", [D, D], F16, kind="ExternalInput")
    out = nc.dram_tensor("out", [NSH, D], F32, kind="ExternalOutput")

    with tile.TileContext(nc) as tc, ExitStack() as ctx:
        nc.gpsimd.load_library(mlp)
        const = ctx.enter_context(tc.tile_pool(name="const", bufs=1))
        idxp = ctx.enter_context(tc.tile_pool(name="idxp", bufs=1))
        xp = ctx.enter_context(tc.tile_pool(name="xp", bufs=1))
        gp = ctx.enter_context(tc.tile_pool(name="gp", bufs=ABUFS))
        sp = ctx.enter_context(tc.tile_pool(name="sp", bufs=6))
        ep = ctx.enter_context(tc.tile_pool(name="ep", bufs=3))
        ps_acc = ctx.enter_context(tc.tile_pool(name="psacc", bufs=4, space="PSUM"))
        ps_mm = ctx.enter_context(tc.tile_pool(name="psmm", bufs=2, space="PSUM"))
        dram = ctx.enter_context(tc.tile_pool(name="dram", bufs=1, space="DRAM"))

        m_t = const.tile([D, D], F16)
        nc.sync.dma_start(m_t[:], m_arr[:, :])
        zero16 = const.tile([P, P], F16)
        nc.vector.memset(zero16[:], 0.0)

        idx1_t = idxp.tile([P, T1 * 8], I16)
        idx2_t = idxp.tile([P, T2 * 8], I16)
        nc.sync.dma_start(idx1_t[:], idx1[:, :])
        nc.sync.dma_start(idx2_t[:], idx2[:, :])

        x0_t = xp.tile([D, NSH], F16, tag="x0")

        # ---- cast X shard f32 -> f16 into DRAM (gather table) ----
        # gpsimd DMAs can cast; one DRAM->DRAM converting copy.
        xsh16 = dram.tile([NSH, D], F16)
        nc.gpsimd.dma_start(xsh16[:], xsh.ap()[:, :])

        xe_part = dram.tile([E_PAD, D], F16)
        xe_full = dram.tile([E_PAD, D], F16)
        # Chunked [j, b, d] layouts: per AR chunk c (blocks [lo,hi)), row
        # base_c + j*(hi-lo) + (b-lo).  Writes batch wb blocks into 2KB+
        # per-partition descriptors; gather indices are remapped host-side.
        bnds = _ar_bounds(cfg)
        nch = cfg.ar_chunks
        chunk_of_block = {}
        chunk_base = []
        base = 0
        for ci in range(nch):
            lo, hi = bnds[ci], bnds[ci + 1]
            chunk_base.append(base)
            for b in range(lo, hi):
                chunk_of_block[b] = ci
            base += P * (hi - lo)
        xe_views = []
        for ci in range(nch):
            lo, hi = bnds[ci], bnds[ci + 1]
            v = xe_part[chunk_base[ci]:chunk_base[ci] + P * (hi - lo), :]
            xe_views.append(v.rearrange("(j w) d -> j (w d)", j=P))
        out_j = out.ap().rearrange("(j w) d -> j (w d)", j=P)

        qn = 0

        def run_phase(T, pairs, per_block, nblocks, idx_t, src_dram, gtag,
                      emit_block, post_arena=None):
            nonlocal qn
            n_arenas = -(-T // AT)
            arena_tiles = {}

            def tile_ref(t):
                a, r = divmod(t, AT)
                q, i = divmod(r, TPC)
                return arena_tiles[a][q][:, i, :]

            done_in = [[] for _ in range(n_arenas)]
            for b in range(nblocks):
                if per_block[b]:
                    last_t = max(pairs[p][0] for p in per_block[b])
                    done_in[min(last_t // AT, n_arenas - 1)].append(b)
                else:
                    done_in[0].append(b)

            for a in range(n_arenas):
                aps = []
                for q in range(NQ):
                    t0 = a * AT + q * TPC
                    ntiles = min(TPC, max(0, T - t0))
                    g_t = gp.tile([P, TPC, P], F16, tag=f"{gtag}{q}")
                    aps.append(g_t)
                    if ntiles > 0:
                        L = ntiles * P
                        nc.gpsimd.dma_gather(
                            g_t[:, :ntiles, :], src_dram[:, :],
                            idx_t[:, t0 * 8:t0 * 8 + L // 16], L, L, D,
                            single_packet=False, queue_num=qn % NQ)
                        qn += 1
                arena_tiles[a] = aps
                for b in done_in[a]:
                    emit_block(b, tile_ref)
                if post_arena and a in post_arena:
                    post_arena[a]()
                arena_tiles.pop(a - ABUFS + 1, None)

        # ---- phase 1: nodes -> hyperedges ----
        wb1 = {"buf": None, "start": -1, "n": 0}

        def flush1():
            if wb1["buf"] is not None and wb1["n"] > 0:
                bs = wb1["start"]
                ci = chunk_of_block[bs]
                c0 = (bs - bnds[ci]) * D
                nc.sync.dma_start(xe_views[ci][:, c0:c0 + wb1["n"] * D],
                                  wb1["buf"][:, :wb1["n"], :])
            wb1["buf"] = None
            wb1["n"] = 0

        def emit_block1(b, tile_ref):
            plist = per_block1[b]
            if wb1["buf"] is None:
                wb1["buf"] = ep.tile([P, cfg.wb, P], F16, tag="xeo", name="xeo_b")
                wb1["start"] = b
                wb1["n"] = 0
            xe_o = wb1["buf"][:, wb1["n"], :]
            wb1["n"] += 1
            if not plist:
                nc.vector.tensor_copy(xe_o, zero16[:])
            else:
                nchain = len(plist)
                p0 = plist[0]
                s_blk = sp.tile([P, nchain * P], F16, tag="s1b",
                                padded_shape=[P, MAXC1 * P])
                nc.sync.dma_start(s_blk[:], s1x[:, p0 * P:(p0 + nchain) * P])
                acc = ps_acc.tile([P, P], F32, tag="acc", space="PSUM")
                for j, p in enumerate(plist):
                    t, _b = pairs1[p]
                    nc.tensor.matmul(acc[:], lhsT=s_blk[:, j * P:(j + 1) * P],
                                     rhs=tile_ref(t),
                                     start=(j == 0), stop=(j == nchain - 1))
                nc.scalar.copy(xe_o, acc[:])
            if wb1["n"] == cfg.wb or b + 1 in bnds:
                flush1()

        # AllReduce chunk plan (chunk slices are contiguous rows in the
        # chunked [j, b, d] layout).
        n_arenas1 = -(-T1 // AT)

        def chunk_done_arena(hi_block):
            last = 0
            for b in range(hi_block):
                if per_block1[b]:
                    last = max(last, pairs1[per_block1[b][-1]][0])
            return min(last // AT, n_arenas1 - 1)

        post1 = {}
        skip_cc = bool(os.environ.get("K_SKIP_CC"))
        if not skip_cc:
            for i in range(nch - 1):
                lo = chunk_base[i]
                hi = chunk_base[i + 1] if i + 1 < nch else E_PAD

                def mk(lo=lo, hi=hi):
                    def f():
                        nc.gpsimd.collective_compute(
                            "AllReduce", mybir.AluOpType.add,
                            replica_groups=[list(range(cfg.ncores))],
                            ins=[xe_part[lo:hi, :].opt()],
                            outs=[xe_full[lo:hi, :].opt()])
                    return f
                # +2 arenas of slack so the AR head-wait (chunk writes) is
                # already satisfied and barely stalls the gather stream.
                a_at = min(chunk_done_arena(bnds[i + 1]) + 2, n_arenas1 - 1)
                post1[a_at] = mk()

        run_phase(T1, pairs1, per_block1, NB_E, idx1_t, xsh16, "g1",
                  emit_block1, post_arena=post1)
        flush1()

        # x0 load+cast between phases (needed for phase 2 only; overlaps AR)
        nc.gpsimd.dma_start(x0_t[:], x0t[:, :])  # SWDGE cast f32->f16
        nc.vector.tensor_scalar(out=x0_t[:], in0=x0_t[:], scalar1=float(alpha),
                                scalar2=None, op0=mybir.AluOpType.mult)

        if skip_cc:
            nc.gpsimd.dma_start(xe_full[:], xe_part[:])
        else:
            lo, hi = chunk_base[nch - 1], E_PAD
            nc.gpsimd.collective_compute(
                "AllReduce", mybir.AluOpType.add,
                replica_groups=[list(range(cfg.ncores))],
                ins=[xe_part[lo:hi, :].opt()], outs=[xe_full[lo:hi, :].opt()])

        # ---- phase 2: hyperedges -> nodes, epilogue ----
        wb2 = {"buf": None, "start": -1, "n": 0}

        def flush2():
            if wb2["buf"] is not None and wb2["n"] > 0:
                c0 = wb2["start"] * D
                nc.sync.dma_start(out_j[:, c0:c0 + wb2["n"] * D],
                                  wb2["buf"][:, :wb2["n"], :])
            wb2["buf"] = None
            wb2["n"] = 0

        def emit_block2(b, tile_ref):
            plist = per_block2[b]
            xiT = ep.tile([P, P], F16, tag="xiT")
            if not plist:
                nc.vector.tensor_copy(xiT[:], x0_t[:, b * P:(b + 1) * P])
            else:
                nchain = len(plist)
                p0 = plist[0]
                s_blk = sp.tile([P, nchain * P], F16, tag="s2b",
                                padded_shape=[P, MAXC2 * P])
                nc.sync.dma_start(s_blk[:], s2x[:, p0 * P:(p0 + nchain) * P])
                acc = ps_acc.tile([P, P], F32, tag="acc", space="PSUM")
                for j, p in enumerate(plist):
                    t, _b = pairs2[p]
                    nc.tensor.matmul(acc[:], lhsT=tile_ref(t),
                                     rhs=s_blk[:, j * P:(j + 1) * P],
                                     start=(j == 0), stop=(j == nchain - 1))
                nc.vector.tensor_tensor(out=xiT[:], in0=acc[:],
                                        in1=x0_t[:, b * P:(b + 1) * P],
                                        op=mybir.AluOpType.add)
            mm = ps_mm.tile([P, P], F32, tag="mm", space="PSUM")
            nc.tensor.matmul(mm[:], lhsT=xiT[:], rhs=m_t[:], start=True, stop=True)
            if wb2["buf"] is None:
                wb2["buf"] = ep.tile([P, cfg.wb, P], F32, tag="outo", name="outo_b")
                wb2["start"] = b
                wb2["n"] = 0
            nc.scalar.copy(wb2["buf"][:, wb2["n"], :], mm[:])
            wb2["n"] += 1
            if wb2["n"] == cfg.wb:
                flush2()

        run_phase(T2, pairs2, per_block2, NB_V, idx2_t, xe_full, "g2",
                  emit_block2)
        flush2()

    if compile:
        nc.compile()
    _PROGRAM_CACHE[key] = nc
    return nc


def build_in_maps(inputs, cfg=CFG):
    """Host-side sharding + index preprocessing."""
    D = cfg.d
    NSH, NB_V, NB_E = cfg.nsh, cfg.nb_v, cfg.nb_e

    X = np.asarray(inputs["X"], np.float32)
    X0 = np.asarray(inputs["X0"], np.float32)
    degE = np.asarray(inputs["degE"], np.float32).reshape(-1)
    degV = np.asarray(inputs["degV"], np.float32).reshape(-1)
    alpha = float(np.asarray(inputs["alpha"]).reshape(-1)[0])
    beta = float(np.asarray(inputs["beta"]).reshape(-1)[0])
    W = np.asarray(inputs["W_w"], np.float32)
    g1_src = np.asarray(inputs["g1_src"]).astype(np.int64)
    g1_dst = np.asarray(inputs["g1_dst"]).astype(np.int64)
    g2_src = np.asarray(inputs["g2_src"]).astype(np.int64)
    g2_dst = np.asarray(inputs["g2_dst"]).astype(np.int64)

    M = (1.0 - beta) * np.eye(D, dtype=np.float32) + beta * W
    m_arr = np.ascontiguousarray(M.T).astype(np.float16)

    degE_pad = np.zeros(cfg.e_pad, np.float32)
    degE_pad[:cfg.n_edges] = degE

    X_pad = np.zeros((cfg.n_pad, D), np.float32)
    X_pad[:cfg.n_nodes] = X
    X0_pad = np.zeros((cfg.n_pad, D), np.float32)
    X0_pad[:cfg.n_nodes] = X0

    core_sets = []
    cnt1 = np.zeros(NB_E, np.int64)
    cnt2 = np.zeros(NB_V, np.int64)
    for c in range(cfg.ncores):
        lo, hi = c * NSH, (c + 1) * NSH
        m1 = (g1_src >= lo) & (g1_src < hi)
        m2 = (g2_dst >= lo) & (g2_dst < hi)
        s1, d1 = g1_src[m1] - lo, g1_dst[m1]
        s2, d2 = g2_src[m2], g2_dst[m2] - lo
        core_sets.append((s1, d1, s2, d2))
        np.maximum(cnt1, np.bincount(d1 // P, minlength=NB_E), out=cnt1)
        np.maximum(cnt2, np.bincount(d2 // P, minlength=NB_V), out=cnt2)

    xe_row = _xe_row_of_edge(cfg)
    off1, T1, pairs1, pb1 = _common_layout(cnt1)
    off2, T2, pairs2, pb2 = _common_layout(cnt2)
    sched1 = (pairs1, pb1, T1)
    sched2 = (pairs2, pb2, T2)

    in_maps = []
    for c in range(cfg.ncores):
        lo = c * NSH
        s1, d1, s2, d2 = core_sets[c]
        i1, sx1 = _fill_core(s1, d1, degE_pad, None, off1, T1, pairs1, NB_E)
        i2, sx2 = _fill_core(xe_row[s2], d2, None, degV[d2 + lo] * (1.0 - alpha),
                             off2, T2, pairs2, NB_V)
        in_maps.append({
            "xsh": np.ascontiguousarray(X_pad[lo:lo + NSH]),
            "x0t": np.ascontiguousarray(X0_pad[lo:lo + NSH].T),
            "idx1": _pack_idx(i1, T1),
            "idx2": _pack_idx(i2, T2),
            "s1x": sx1,
            "s2x": sx2,
            "m_arr": m_arr,
        })
    return in_maps, (sched1, sched2), alpha


def _enable_axon_trace_hook():
    """Best-effort: register the NTFF profile hook so BASS_TRACE=1 works."""
    try:
        import sys, types
        import antenv  # noqa: F401
        if "antenv.axon_hooks" not in sys.modules:
            from trn_agent_boot.trn_boot import _ntff_profile_via_ctypes
            hook = _ntff_profile_via_ctypes("/opt/axon/libaxon_pjrt.so")
            hm = types.ModuleType("antenv.axon_hooks")
            hm.get_axon_ntff_profile_hook = lambda: hook
            hm.set_axon_ntff_profile_hook = lambda h: None
            sys.modules["antenv.axon_hooks"] = hm
        import concourse.bass_utils as bu
        bu.upload_artifacts = lambda tmpdir: "local://" + tmpdir
    except Exception:
        pass


LAST_EXEC_TIME_NS = None


def kernel(**inputs):
    global LAST_EXEC_TIME_NS
    cfg = CFG
    in_maps, scheds, alpha = build_in_maps(inputs, cfg)

    if os.environ.get("BASS_TRACE"):
        _enable_axon_trace_hook()

    nc = build_program(scheds[0], scheds[1], alpha, cfg)
    res = run_bass_kernel_spmd(nc, in_maps, core_ids=list(range(cfg.ncores)))
    LAST_EXEC_TIME_NS = res.exec_time_ns

    W = cfg.nsh // P
    outs = []
    for c in range(cfg.ncores):
        o = res.results[c]["out"].reshape(P, W, cfg.d).transpose(1, 0, 2)
        outs.append(o.reshape(cfg.nsh, cfg.d))
    out = np.concatenate(outs, axis=0)
    return np.ascontiguousarray(out[:cfg.n_nodes]).astype(np.float32)


# revision 24
# speedup vs baseline: 1.0483x; 1.0483x over previous
"""Trainium2 Bass kernel for DGL HyperGCNII conv (hypergraph message passing).

Computation (reference):
    Xe = segment_sum(X[g1_src], g1_dst, E) * degE          # nodes -> hyperedges
    Xv = segment_sum(Xe[g2_src], g2_dst, N) * degV         # hyperedges -> nodes
    Xi = (1-a)*Xv + a*X0
    out = (1-b)*Xi + b*(Xi @ W.T)

Strategy (8 NeuronCores, vertex-cut graph parallelism):
- Shard nodes across cores.  Each phase's nnz are globally sorted by
  destination and packed into 128-slot tiles; per-block counts are padded to
  the max across cores so the compiled schedule is core-uniform (SPMD).
- Gathers run as SWDGE dma_gather calls round-robined over the 4 SWDGE
  queues (4 Q7 core-pairs emit descriptors in parallel).  Gather tiles are
  grouped into large rotating ARENAS (4 calls per arena, one per queue) so
  the descriptor rings stay deep; random 256B HBM reads are latency bound,
  so ring depth is what buys aggregate drain throughput.
- Segment-sum via one-hot selection matmuls.  The one-hot S tiles are pure
  index metadata and are precomputed host-side (degE / degV*(1-alpha) folds
  included) and streamed from HBM per block -- building them on DVE/ACT
  on-chip stalls badly on SBUF bank conflicts with the gather drain.
- AllReduce (fp16, 2 chunks, triggered from the Scalar engine so the first
  chunk overlaps the phase-1 gather tail) of Xe partials across 8 cores.
- Phase 2 accumulates transposed (Xv^T), adds a*X0^T, applies
  M = (1-b)I + b*W via a second matmul which also un-transposes, writes out.

All indices / one-hot selection matrices are precomputed host-side as int16 /
f16 metadata (index-only preprocessing); data math happens on device.
"""

import hashlib
import os
import numpy as np
from contextlib import ExitStack
from dataclasses import dataclass

import concourse.bass as bass
import concourse.tile as tile
from concourse import bacc, mybir
from concourse.bass_utils import run_bass_kernel_spmd
from concourse.library_config import mlp

P = 128
F32 = mybir.dt.float32
F16 = mybir.dt.float16
I16 = mybir.dt.int16
NQ = 4   # SWDGE queues (4 Q7 core-pairs)
AT = 48  # tiles per arena (divisible by NQ)
ABUFS = 4


@dataclass(frozen=True)
class Cfg:
    n_nodes: int = 100000
    n_edges: int = 20000
    d: int = 128
    ncores: int = 8
    ar_chunks: int = 3
    sgb: int = 4  # blocks per batched S load
    wb: int = 8   # blocks per batched DRAM write

    @property
    def nb_v(self):
        return -(-self.n_nodes // (self.ncores * P))

    @property
    def nsh(self):
        return self.nb_v * P

    @property
    def n_pad(self):
        return self.nsh * self.ncores

    @property
    def nb_e(self):
        return -(-self.n_edges // P)

    @property
    def e_pad(self):
        return self.nb_e * P


CFG = Cfg()


def _common_layout(cnts):
    """Uniform (across cores) slot-stream layout from per-block padded counts.

    Returns (off[nblocks+1], T, pairs list of (tile, block), per_block).
    """
    nblocks = len(cnts)
    off = np.zeros(nblocks + 1, np.int64)
    np.cumsum(cnts, out=off[1:])
    S = int(off[-1])
    T = max(1, -(-S // P))
    pairs = []
    per_block = [[] for _ in range(nblocks)]
    for b in range(nblocks):
        if cnts[b] == 0:
            continue
        t0 = int(off[b]) // P
        t1 = int(off[b] + cnts[b] - 1) // P
        for t in range(t0, t1 + 1):
            per_block[b].append(len(pairs))
            pairs.append((t, b))
    return off, T, pairs, per_block


def _fill_core(src, dst_local, colw, slotw, off, T, pairs, nblocks):
    """Place one core's nnz into the common layout.

    colw: per-destination column weights [nblocks*128] (phase 1: degE) or
          None; slotw: per-nnz slot weights (phase 2: degV*(1-a)) or None.
    Returns (idx_slots[T*128] int64, sx [128, npairs*128] f16 one-hot tiles).
    """
    dl = np.asarray(dst_local, np.int64)
    order = np.argsort(dl, kind="stable")
    s = np.asarray(src, np.int64)[order]
    dls = dl[order]
    blk = dls // P
    bc = np.bincount(blk, minlength=nblocks)
    bstart = np.zeros(nblocks + 1, np.int64)
    np.cumsum(bc, out=bstart[1:])
    rank = np.arange(len(dls)) - bstart[blk]
    pos = off[blk] + rank
    BIG = np.int64(1) << 40
    idx_slots = np.zeros(T * P, np.int64)
    dl_full = np.full(T * P, BIG)
    w_full = np.ones(T * P, np.float32)
    idx_slots[pos] = s
    dl_full[pos] = dls
    if slotw is not None:
        w_full[:] = 0.0
        w_full[pos] = np.asarray(slotw, np.float32)[order]
    blk_full = dl_full // P

    npairs = len(pairs)
    tile_of_pair = np.asarray([t for t, _ in pairs], np.int64)
    blk_of_pair = np.asarray([b for _, b in pairs], np.int64)
    sx = np.zeros((npairs, P, P), np.float16)
    slot_mat = dl_full.reshape(T, P)
    blk_mat = blk_full.reshape(T, P)
    w_mat = w_full.reshape(T, P)
    for i in range(npairs):
        t, b = tile_of_pair[i], blk_of_pair[i]
        m = blk_mat[t] == b
        if not m.any():
            continue
        cols = (slot_mat[t][m] - b * P).astype(np.int64)
        vals = w_mat[t][m].astype(np.float32)
        if colw is not None:
            vals = vals * colw[b * P + cols]
        sx[i, np.nonzero(m)[0], cols] = vals.astype(np.float16)
    sx = np.ascontiguousarray(sx.transpose(1, 0, 2).reshape(P, npairs * P))
    return idx_slots, sx


def _pack_idx(idx_slots, T):
    """[T*128] slot ids -> SWDGE 16-wrap [128, T*8] int16."""
    cols = []
    for t in range(T):
        flat = idx_slots[t * P:(t + 1) * P].astype(np.int16)
        wrap = flat.reshape(-1, 16).T          # [16, 8]
        cols.append(np.tile(wrap, (8, 1)))     # [128, 8]
    return np.ascontiguousarray(np.concatenate(cols, axis=1))


def _ar_bounds(cfg):
    NB_E = cfg.nb_e
    if cfg.ar_chunks == 3:
        return [0, round(0.40 * NB_E), round(0.70 * NB_E), NB_E]
    return [round(i * NB_E / cfg.ar_chunks) for i in range(cfg.ar_chunks + 1)]


def _xe_row_of_edge(cfg):
    """Edge id -> row in the chunked [j, b, d] xe layout."""
    bnds = _ar_bounds(cfg)
    e = np.arange(cfg.e_pad, dtype=np.int64)
    b = e // P
    j = e % P
    row = np.zeros(cfg.e_pad, np.int64)
    base = 0
    for c in range(len(bnds) - 1):
        lo, hi = bnds[c], bnds[c + 1]
        w = hi - lo
        m = (b >= lo) & (b < hi)
        row[m] = base + j[m] * w + (b[m] - lo)
        base += P * w
    return row


_PROGRAM_CACHE = {}


def _schedule_hash(sched1, sched2, alpha):
    h = hashlib.sha1()
    for pairs, per_block, T in (sched1, sched2):
        h.update(np.int64(T).tobytes())
        h.update(np.asarray([p for pr in pairs for p in pr], np.int64).tobytes())
        for pb in per_block:
            h.update(np.asarray(pb + [-1], np.int64).tobytes())
    h.update(np.float64(alpha).tobytes())
    return h.hexdigest()


def build_program(sched1, sched2, alpha, cfg=CFG, compile=True):
    key = _schedule_hash(sched1, sched2, alpha)
    if key in _PROGRAM_CACHE:
        return _PROGRAM_CACHE[key]

    D = cfg.d
    NSH, NB_V, NB_E, E_PAD = cfg.nsh, cfg.nb_v, cfg.nb_e, cfg.e_pad
    pairs1, per_block1, T1 = sched1
    pairs2, per_block2, T2 = sched2
    NP1, NP2 = len(pairs1), len(pairs2)
    MAXC1 = max((len(x) for x in per_block1 if x), default=1)
    MAXC2 = max((len(x) for x in per_block2 if x), default=1)
    TPC = AT // NQ

    nc = bacc.Bacc("TRN2", target_bir_lowering=False, debug=False,
                   num_devices=cfg.ncores, num_swdge_queues=NQ)

    xsh = nc.dram_tensor("xsh", [NSH, D], F32, kind="ExternalInput")
    x0t = nc.dram_tensor("x0t", [D, NSH], F32, kind="ExternalInput")
    idx1 = nc.dram_tensor("idx1", [P, T1 * 8], I16, kind="ExternalInput")
    idx2 = nc.dram_tensor("idx2", [P, T2 * 8], I16, kind="ExternalInput")
    s1x = nc.dram_tensor("s1x", [P, NP1 * P], F16, kind="ExternalInput")
    s2x = nc.dram_tensor("s2x", [P, NP2 * P], F16, kind="ExternalInput")
    m_arr = nc.dram_tensor("m_arr", [D, D], F16, kind="ExternalInput")
    out = nc.dram_tensor("out", [NSH, D], F32, kind="ExternalOutput")

    with tile.TileContext(nc) as tc, ExitStack() as ctx:
        nc.gpsimd.load_library(mlp)
        const = ctx.enter_context(tc.tile_pool(name="const", bufs=1))
        idxp = ctx.enter_context(tc.tile_pool(name="idxp", bufs=1))
        xp = ctx.enter_context(tc.tile_pool(name="xp", bufs=1))
        gp = ctx.enter_context(tc.tile_pool(name="gp", bufs=ABUFS))
        sp = ctx.enter_context(tc.tile_pool(name="sp", bufs=2))
        ep = ctx.enter_context(tc.tile_pool(name="ep", bufs=3))
        ps_acc = ctx.enter_context(tc.tile_pool(name="psacc", bufs=4, space="PSUM"))
        ps_mm = ctx.enter_context(tc.tile_pool(name="psmm", bufs=2, space="PSUM"))
        dram = ctx.enter_context(tc.tile_pool(name="dram", bufs=1, space="DRAM"))

        m_t = const.tile([D, D], F16)
        nc.sync.dma_start(m_t[:], m_arr[:, :])
        zero16 = const.tile([P, P], F16)
        nc.vector.memset(zero16[:], 0.0)

        idx1_t = idxp.tile([P, T1 * 8], I16)
        idx2_t = idxp.tile([P, T2 * 8], I16)
        nc.sync.dma_start(idx1_t[:], idx1[:, :])
        nc.sync.dma_start(idx2_t[:], idx2[:, :])

        x0_t = xp.tile([D, NSH], F16, tag="x0")

        # ---- cast X shard f32 -> f16 into DRAM (gather table) ----
        # gpsimd DMAs can cast; one DRAM->DRAM converting copy.
        xsh16 = dram.tile([NSH, D], F16)
        nc.gpsimd.dma_start(xsh16[:], xsh.ap()[:, :])

        xe_part = dram.tile([E_PAD, D], F16)
        xe_full = dram.tile([E_PAD, D], F16)
        # Chunked [j, b, d] layouts: per AR chunk c (blocks [lo,hi)), row
        # base_c + j*(hi-lo) + (b-lo).  Writes batch wb blocks into 2KB+
        # per-partition descriptors; gather indices are remapped host-side.
        bnds = _ar_bounds(cfg)
        nch = cfg.ar_chunks
        chunk_of_block = {}
        chunk_base = []
        base = 0
        for ci in range(nch):
            lo, hi = bnds[ci], bnds[ci + 1]
            chunk_base.append(base)
            for b in range(lo, hi):
                chunk_of_block[b] = ci
            base += P * (hi - lo)
        xe_views = []
        for ci in range(nch):
            lo, hi = bnds[ci], bnds[ci + 1]
            v = xe_part[chunk_base[ci]:chunk_base[ci] + P * (hi - lo), :]
            xe_views.append(v.rearrange("(j w) d -> j (w d)", j=P))
        out_j = out.ap().rearrange("(j w) d -> j (w d)", j=P)

        qn = 0

        def run_phase(T, pairs, per_block, nblocks, idx_t, src_dram, gtag,
                      emit_block, post_arena=None):
            nonlocal qn
            n_arenas = -(-T // AT)
            arena_tiles = {}

            def tile_ref(t):
                a, r = divmod(t, AT)
                q, i = divmod(r, TPC)
                return arena_tiles[a][q][:, i, :]

            done_in = [[] for _ in range(n_arenas)]
            for b in range(nblocks):
                if per_block[b]:
                    last_t = max(pairs[p][0] for p in per_block[b])
                    done_in[min(last_t // AT, n_arenas - 1)].append(b)
                else:
                    done_in[0].append(b)

            for a in range(n_arenas):
                aps = []
                for q in range(NQ):
                    t0 = a * AT + q * TPC
                    ntiles = min(TPC, max(0, T - t0))
                    g_t = gp.tile([P, TPC, P], F16, tag=f"{gtag}{q}")
                    aps.append(g_t)
                    if ntiles > 0:
                        L = ntiles * P
                        nc.gpsimd.dma_gather(
                            g_t[:, :ntiles, :], src_dram[:, :],
                            idx_t[:, t0 * 8:t0 * 8 + L // 16], L, L, D,
                            single_packet=False, queue_num=qn % NQ)
                        qn += 1
                arena_tiles[a] = aps
                for b in done_in[a]:
                    emit_block(b, tile_ref)
                if post_arena and a in post_arena:
                    post_arena[a]()
                arena_tiles.pop(a - ABUFS + 1, None)

        # ---- phase 1: nodes -> hyperedges ----
        # S-load groups: sgb consecutive non-empty blocks share one DMA.
        def make_sgroups(per_block, nblocks):
            groups = []
            cur = []
            for b in range(nblocks):
                if not per_block[b]:
                    continue
                cur.append(b)
                if len(cur) == cfg.sgb:
                    groups.append(cur)
                    cur = []
            if cur:
                groups.append(cur)
            binfo = {}
            gmax = 1
            for gi, blocks in enumerate(groups):
                p0 = per_block[blocks[0]][0]
                p1 = per_block[blocks[-1]][-1] + 1
                gmax = max(gmax, p1 - p0)
                for b in blocks:
                    binfo[b] = (gi, p0, p1, b == blocks[0])
            return binfo, gmax

        sg1, SG1MAX = make_sgroups(per_block1, NB_E)
        sg2, SG2MAX = make_sgroups(per_block2, NB_V)
        sg1_tiles = {}
        sg2_tiles = {}

        wb1 = {"buf": None, "start": -1, "n": 0}

        def flush1():
            if wb1["buf"] is not None and wb1["n"] > 0:
                bs = wb1["start"]
                ci = chunk_of_block[bs]
                c0 = (bs - bnds[ci]) * D
                nc.sync.dma_start(xe_views[ci][:, c0:c0 + wb1["n"] * D],
                                  wb1["buf"][:, :wb1["n"], :])
            wb1["buf"] = None
            wb1["n"] = 0

        def emit_block1(b, tile_ref):
            plist = per_block1[b]
            if wb1["buf"] is None:
                wb1["buf"] = ep.tile([P, cfg.wb, P], F16, tag="xeo", name="xeo_b")
                wb1["start"] = b
                wb1["n"] = 0
            xe_o = wb1["buf"][:, wb1["n"], :]
            wb1["n"] += 1
            if not plist:
                nc.vector.tensor_copy(xe_o, zero16[:])
            else:
                gi, gp0, gp1, first = sg1[b]
                if first:
                    s_blk = sp.tile([P, (gp1 - gp0) * P], F16, tag="s1b",
                                    padded_shape=[P, SG1MAX * P], name="s1g")
                    nc.sync.dma_start(s_blk[:], s1x[:, gp0 * P:gp1 * P])
                    sg1_tiles.clear()
                    sg1_tiles[gi] = s_blk
                s_blk = sg1_tiles[gi]
                acc = ps_acc.tile([P, P], F32, tag="acc", space="PSUM")
                nchain = len(plist)
                for j, p in enumerate(plist):
                    t, _b = pairs1[p]
                    co = (p - gp0) * P
                    nc.tensor.matmul(acc[:], lhsT=s_blk[:, co:co + P],
                                     rhs=tile_ref(t),
                                     start=(j == 0), stop=(j == nchain - 1))
                nc.scalar.copy(xe_o, acc[:])
            if wb1["n"] == cfg.wb or b + 1 in bnds:
                flush1()

        # AllReduce chunk plan (chunk slices are contiguous rows in the
        # chunked [j, b, d] layout).
        n_arenas1 = -(-T1 // AT)

        def chunk_done_arena(hi_block):
            last = 0
            for b in range(hi_block):
                if per_block1[b]:
                    last = max(last, pairs1[per_block1[b][-1]][0])
            return min(last // AT, n_arenas1 - 1)

        post1 = {}
        skip_cc = bool(os.environ.get("K_SKIP_CC"))
        if not skip_cc:
            for i in range(nch - 1):
                lo = chunk_base[i]
                hi = chunk_base[i + 1] if i + 1 < nch else E_PAD

                def mk(lo=lo, hi=hi):
                    def f():
                        nc.gpsimd.collective_compute(
                            "AllReduce", mybir.AluOpType.add,
                            replica_groups=[list(range(cfg.ncores))],
                            ins=[xe_part[lo:hi, :].opt()],
                            outs=[xe_full[lo:hi, :].opt()])
                    return f
                # +2 arenas of slack so the AR head-wait (chunk writes) is
                # already satisfied and barely stalls the gather stream.
                a_at = min(chunk_done_arena(bnds[i + 1]) + 2, n_arenas1 - 1)
                post1[a_at] = mk()

        run_phase(T1, pairs1, per_block1, NB_E, idx1_t, xsh16, "g1",
                  emit_block1, post_arena=post1)
        flush1()

        # x0 load+cast between phases (needed for phase 2 only; overlaps AR)
        nc.gpsimd.dma_start(x0_t[:], x0t[:, :])  # SWDGE cast f32->f16
        nc.vector.tensor_scalar(out=x0_t[:], in0=x0_t[:], scalar1=float(alpha),
                                scalar2=None, op0=mybir.AluOpType.mult)

        if skip_cc:
            nc.gpsimd.dma_start(xe_full[:], xe_part[:])
        else:
            lo, hi = chunk_base[nch - 1], E_PAD
            nc.gpsimd.collective_compute(
                "AllReduce", mybir.AluOpType.add,
                replica_groups=[list(range(cfg.ncores))],
                ins=[xe_part[lo:hi, :].opt()], outs=[xe_full[lo:hi, :].opt()])

        # ---- phase 2: hyperedges -> nodes, epilogue ----
        wb2 = {"buf": None, "start": -1, "n": 0}

        def flush2():
            if wb2["buf"] is not None and wb2["n"] > 0:
                c0 = wb2["start"] * D
                nc.sync.dma_start(out_j[:, c0:c0 + wb2["n"] * D],
                                  wb2["buf"][:, :wb2["n"], :])
            wb2["buf"] = None
            wb2["n"] = 0

        def emit_block2(b, tile_ref):
            plist = per_block2[b]
            xiT = ep.tile([P, P], F16, tag="xiT")
            if not plist:
                nc.vector.tensor_copy(xiT[:], x0_t[:, b * P:(b + 1) * P])
            else:
                gi, gp0, gp1, first = sg2[b]
                if first:
                    s_blk = sp.tile([P, (gp1 - gp0) * P], F16, tag="s2b",
                                    padded_shape=[P, SG2MAX * P], name="s2g")
                    nc.sync.dma_start(s_blk[:], s2x[:, gp0 * P:gp1 * P])
                    sg2_tiles.clear()
                    sg2_tiles[gi] = s_blk
                s_blk = sg2_tiles[gi]
                acc = ps_acc.tile([P, P], F32, tag="acc", space="PSUM")
                nchain = len(plist)
                for j, p in enumerate(plist):
                    t, _b = pairs2[p]
                    co = (p - gp0) * P
                    nc.tensor.matmul(acc[:], lhsT=tile_ref(t),
                                     rhs=s_blk[:, co:co + P],
                                     start=(j == 0), stop=(j == nchain - 1))
                nc.vector.tensor_tensor(out=xiT[:], in0=acc[:],
                                        in1=x0_t[:, b * P:(b + 1) * P],
                                        op=mybir.AluOpType.add)
            mm = ps_mm.tile([P, P], F32, tag="mm", space="PSUM")
            nc.tensor.matmul(mm[:], lhsT=xiT[:], rhs=m_t[:], start=True, stop=True)
            if wb2["buf"] is None:
                wb2["buf"] = ep.tile([P, cfg.wb, P], F32, tag="outo", name="outo_b")
                wb2["start"] = b
                wb2["n"] = 0
            nc.scalar.copy(wb2["buf"][:, wb2["n"], :], mm[:])
            wb2["n"] += 1
            if wb2["n"] == cfg.wb:
                flush2()

        run_phase(T2, pairs2, per_block2, NB_V, idx2_t, xe_full, "g2",
                  emit_block2)
        flush2()

    if compile:
        nc.compile()
    _PROGRAM_CACHE[key] = nc
    return nc


def build_in_maps(inputs, cfg=CFG):
    """Host-side sharding + index preprocessing."""
    D = cfg.d
    NSH, NB_V, NB_E = cfg.nsh, cfg.nb_v, cfg.nb_e

    X = np.asarray(inputs["X"], np.float32)
    X0 = np.asarray(inputs["X0"], np.float32)
    degE = np.asarray(inputs["degE"], np.float32).reshape(-1)
    degV = np.asarray(inputs["degV"], np.float32).reshape(-1)
    alpha = float(np.asarray(inputs["alpha"]).reshape(-1)[0])
    beta = float(np.asarray(inputs["beta"]).reshape(-1)[0])
    W = np.asarray(inputs["W_w"], np.float32)
    g1_src = np.asarray(inputs["g1_src"]).astype(np.int64)
    g1_dst = np.asarray(inputs["g1_dst"]).astype(np.int64)
    g2_src = np.asarray(inputs["g2_src"]).astype(np.int64)
    g2_dst = np.asarray(inputs["g2_dst"]).astype(np.int64)

    M = (1.0 - beta) * np.eye(D, dtype=np.float32) + beta * W
    m_arr = np.ascontiguousarray(M.T).astype(np.float16)

    degE_pad = np.zeros(cfg.e_pad, np.float32)
    degE_pad[:cfg.n_edges] = degE

    X_pad = np.zeros((cfg.n_pad, D), np.float32)
    X_pad[:cfg.n_nodes] = X
    X0_pad = np.zeros((cfg.n_pad, D), np.float32)
    X0_pad[:cfg.n_nodes] = X0

    core_sets = []
    cnt1 = np.zeros(NB_E, np.int64)
    cnt2 = np.zeros(NB_V, np.int64)
    for c in range(cfg.ncores):
        lo, hi = c * NSH, (c + 1) * NSH
        m1 = (g1_src >= lo) & (g1_src < hi)
        m2 = (g2_dst >= lo) & (g2_dst < hi)
        s1, d1 = g1_src[m1] - lo, g1_dst[m1]
        s2, d2 = g2_src[m2], g2_dst[m2] - lo
        core_sets.append((s1, d1, s2, d2))
        np.maximum(cnt1, np.bincount(d1 // P, minlength=NB_E), out=cnt1)
        np.maximum(cnt2, np.bincount(d2 // P, minlength=NB_V), out=cnt2)

    xe_row = _xe_row_of_edge(cfg)
    off1, T1, pairs1, pb1 = _common_layout(cnt1)
    off2, T2, pairs2, pb2 = _common_layout(cnt2)
    sched1 = (pairs1, pb1, T1)
    sched2 = (pairs2, pb2, T2)

    in_maps = []
    for c in range(cfg.ncores):
        lo = c * NSH
        s1, d1, s2, d2 = core_sets[c]
        i1, sx1 = _fill_core(s1, d1, degE_pad, None, off1, T1, pairs1, NB_E)
        i2, sx2 = _fill_core(xe_row[s2], d2, None, degV[d2 + lo] * (1.0 - alpha),
                             off2, T2, pairs2, NB_V)
        in_maps.append({
            "xsh": np.ascontiguousarray(X_pad[lo:lo + NSH]),
            "x0t": np.ascontiguousarray(X0_pad[lo:lo + NSH].T),
            "idx1": _pack_idx(i1, T1),
            "idx2": _pack_idx(i2, T2),
            "s1x": sx1,
            "s2x": sx2,
            "m_arr": m_arr,
        })
    return in_maps, (sched1, sched2), alpha


def _enable_axon_trace_hook():
    """Best-effort: register the NTFF profile hook so BASS_TRACE=1 works."""
    try:
        import sys, types
        import antenv  # noqa: F401
        if "antenv.axon_hooks" not in sys.modules:
            from trn_agent_boot.trn_boot import _ntff_profile_via_ctypes
            hook = _ntff_profile_via_ctypes("/opt/axon/libaxon_pjrt.so")
            hm = types.ModuleType("antenv.axon_hooks")
            hm.get_axon_ntff_profile_hook = lambda: hook
            hm.set_axon_ntff_profile_hook = lambda h: None
            sys.modules["antenv.axon_hooks"] = hm
        import concourse.bass_utils as bu
        bu.upload_artifacts = lambda tmpdir: "local://" + tmpdir
    except Exception:
        pass


LAST_EXEC_TIME_NS = None


def kernel(**inputs):
    global LAST_EXEC_TIME_NS
    cfg = CFG
    in_maps, scheds, alpha = build_in_maps(inputs, cfg)

    if os.environ.get("BASS_TRACE"):
        _enable_axon_trace_hook()

    nc = build_program(scheds[0], scheds[1], alpha, cfg)
    res = run_bass_kernel_spmd(nc, in_maps, core_ids=list(range(cfg.ncores)))
    LAST_EXEC_TIME_NS = res.exec_time_ns

    W = cfg.nsh // P
    outs = []
    for c in range(cfg.ncores):
        o = res.results[c]["out"].reshape(P, W, cfg.d).transpose(1, 0, 2)
        outs.append(o.reshape(cfg.nsh, cfg.d))
    out = np.concatenate(outs, axis=0)
    return np.ascontiguousarray(out[:cfg.n_nodes]).astype(np.float32)
